# revision 22
# baseline (speedup 1.0000x reference)
"""Mixture-of-logistics NLL loss (reduction=mean) on 8 Trainium2 NeuronCores.

Math (per row, K=16 mixture components):
    log_prob = ln(num) - ln(den)
    den = sum_k e^{w_k},  num = sum_k e^{w_k} * pdf_k
    pdf = (1 - th^2) * rp / 4,  th = tanh(0.5*(t-loc)*rp),  rp = e^{-ln s}
Kernel accumulates stash = (sum_k e^w | sum_k (1-th^2)*rp*e^w) per row; the
final ACT Ln(scale=0.25) turns the second into ln(num). Output = [128, 2]
per-partition partial sums (sum ln num | sum ln den); host combines.

Sharding: pure data parallel over rows (batch*seq) across 8 cores.

Structure (per core, r=2048 rows/partition; 13 graduated tiles in 7 chunks,
software-pipelined A(h+1)-before-B(h) exactly like the tuned predecessor --
that shape measured best for DMA/engine overlap):
  phase A (chunk): DMA w/loc (bf16-cast SWDGE), Exp(-u)->rp, Exp(w)->ew,
                   diff=t-loc, v=diff*rp, pw=rp*ew, den tree
  phase B (chunk): Tanh, square path, term=(1-th^2)*pw, num tree
  final: two Lns with accum_out.

ACT table economics (walrus gives Ln and Exp different home tables; Tanh and
Square share Exp's): Lns are batched on SUPER-TILES spanning ~2 chunks of
rows -- scale is DMAd into 4 big spans and Ln'd in 2 instructions per span,
so the natural_log<->exp switch happens once per ~2 chunks instead of per
chunk: ~9 table loads instead of 15.

Other engine notes (from tracing):
 - GpSimd tensor ops lock the SBUF port shared with DVE -> SWDGE descgen only.
 - (t - loc) broadcast-sub runs in 1x DVE mode (stride-0 AP); materializing
   t16 via DMA is impossible (descriptor-per-element) and via ACT Copy is
   dominated by giving ACT a Square instead.
 - th^2 path on DVE: ts(-th) 4x, mul(-th,th) 2x, ts(1-x) 4x -- distinct
   operands keep 2x (same-operand th*th would be 1x), no copy needed.
 - row-sum trees run in place on the summed tile's upper lanes (no temps).
 - v/pw go to fresh tiles so DMA-fed input tiles recycle fast; DMA never
   waits on the compute tail.
 - bf16 on-chip everywhere (2x/4x DVE modes); SWDGE casts f32->bf16 in
   flight. Validated rel err ~3e-4 (gate 2e-2).
"""

import numpy as np

import concourse.bacc as bacc
import concourse.mybir as mybir
import concourse.tile as tile
from concourse.tile_rust import add_dep_helper
from concourse.bass_utils import run_bass_kernel_spmd

B, T, K = 16, 131072, 16
N = B * T                 # 2097152 rows total
NCORES = 8
NLOC = N // NCORES        # 262144 rows per core
P = 128                   # SBUF partitions

F32 = mybir.dt.float32
BF16 = mybir.dt.bfloat16
AF = mybir.ActivationFunctionType
OP = mybir.AluOpType


def build_kernel(nloc=NLOC, chunks=None, supers=None, act_square_budget=8):
    """Build the per-core Bass module.

    chunks: tuples of per-tile row counts. supers: row-span sizes for the
    scale/Ln super-tiles (each spanning >=1 chunk; must sum to r).
    """
    p = P
    r = nloc // p             # rows per partition
    if chunks is None:
        chunks = [(32, 64), (96, 192), (192, 192), (192, 192), (192, 192),
                  (192, 192), (128,)]
    if supers is None:
        supers = [384, 768, 768, 128]   # chunks 0-1 | 2-3 | 4-5 | 6
    assert sum(sum(ch) for ch in chunks) == r and nloc % p == 0
    assert sum(supers) == r
    cmax = max(max(ch) for ch in chunks)
    smax = max(supers)

    nc = bacc.Bacc("TRN2", target_bir_lowering=False, debug=False)
    w_d = nc.dram_tensor("w", [nloc, K], F32, kind="ExternalInput")
    loc_d = nc.dram_tensor("loc", [nloc, K], F32, kind="ExternalInput")
    scale_d = nc.dram_tensor("scale", [nloc, K], F32, kind="ExternalInput")
    t_d = nc.dram_tensor("t", [nloc], F32, kind="ExternalInput")
    out_d = nc.dram_tensor("out", [p, 2], F32, kind="ExternalOutput")

    wv = w_d.ap().rearrange("(p r) k -> p r k", p=p)
    lv = loc_d.ap().rearrange("(p r) k -> p r k", p=p)
    sv = scale_d.ap().rearrange("(p r) k -> p r k", p=p)
    tv = t_d.ap().rearrange("(p r) -> p r", p=p)

    acts = []  # every ACT instruction, in required execution order

    def act(*args, **kwargs):
        ins = nc.scalar.activation(*args, **kwargs)
        acts.append(ins)
        return ins

    with tile.TileContext(nc) as tc:
        with (
            tc.tile_pool(name="persist", bufs=1) as pp,
            tc.tile_pool(name="psc", bufs=2) as psc,
            tc.tile_pool(name="pwld", bufs=4) as pwld,
            tc.tile_pool(name="plc", bufs=4) as plc,
            tc.tile_pool(name="prp", bufs=3) as prp,
            tc.tile_pool(name="pv", bufs=5) as pv,
            tc.tile_pool(name="ppw", bufs=5) as ppw,
            tc.tile_pool(name="pc1", bufs=2) as pc1,
            nc.allow_low_precision("bf16 partial sums validated: ~3e-4 rel"),
        ):
            t_all = pp.tile([p, r], BF16)         # targets (bf16 master)
            stash2 = pp.tile([p, 2, r], F32)      # per-row (den | 4*num) sums
            out_sb = pp.tile([p, 2], F32)
            nc.gpsimd.dma_start(out=t_all, in_=tv)

            def tree16(h, dst_slice):
                """Sum h [p, c, 16] bf16 over last axis -> dst [p, c] f32.
                Levels go in place into h's upper lanes (destroys h)."""
                nc.vector.tensor_add(out=h[:, :, 8:16], in0=h[:, :, 0:8],
                                     in1=h[:, :, 8:16])
                nc.vector.tensor_add(out=h[:, :, 4:8], in0=h[:, :, 8:12],
                                     in1=h[:, :, 12:16])
                nc.vector.tensor_add(out=h[:, :, 2:4], in0=h[:, :, 4:6],
                                     in1=h[:, :, 6:8])
                nc.vector.tensor_add(out=dst_slice, in0=h[:, :, 2],
                                     in1=h[:, :, 3])

            # --- super-tile scale loads + batched Lns -----------------------
            su_tiles = {}
            su_starts = []
            o = 0
            for srows in supers:
                su_starts.append(o)
                o += srows

            def emit_super(si):
                srows = supers[si]
                so = su_starts[si]
                u_t = psc.tile([p, smax, K], BF16, tag="sc", name="sc")[:, :srows, :]
                nc.gpsimd.dma_start(out=u_t, in_=sv[:, so:so + srows, :])
                # two Ln instructions per span pipeline with the span's DMA
                h = srows // 2
                act(out=u_t[:, :h, :], in_=u_t[:, :h, :], func=AF.Ln)
                act(out=u_t[:, h:, :], in_=u_t[:, h:, :], func=AF.Ln)
                su_tiles[si] = u_t

            def u_slice(sl):
                # the u (=ln s) view of row-slice sl inside its super tile
                for si, so in enumerate(su_starts):
                    if so <= sl.start and sl.stop <= so + supers[si]:
                        return su_tiles[si][:, sl.start - so:sl.stop - so, :]
                raise AssertionError(f"no super covers {sl}")

            off = 0
            starts = []
            for ch in chunks:
                starts.append(off)
                off += sum(ch)

            sq_left = [act_square_budget]

            def emit_A(ci, ch):
                # ---- phase A of chunk: DMAs, Exp(-u), Exp(w), sub, v, pw,
                #      den tree ----
                tinfo = []
                o = starts[ci]
                for c in ch:
                    sl = slice(o, o + c)
                    o += c
                    w_t = pwld.tile([p, cmax, K], BF16, tag="w", name="wt")[:, :c, :]
                    loc_t = plc.tile([p, cmax, K], BF16, tag="loc", name="loct")[:, :c, :]
                    # SWDGE DMAs cast f32->bf16 in flight
                    nc.gpsimd.dma_start(out=w_t, in_=wv[:, sl, :])
                    nc.gpsimd.dma_start(out=loc_t, in_=lv[:, sl, :])
                    tinfo.append((sl, c, w_t, loc_t))

                binfo = []
                for sl, c, w_t, loc_t in tinfo:
                    rp_t = prp.tile([p, cmax, K], BF16, tag="rp", name="rpt")[:, :c, :]
                    act(out=rp_t, in_=u_slice(sl), func=AF.Exp, scale=-1.0)  # 1/s
                    act(out=w_t, in_=w_t, func=AF.Exp)                 # e^w
                    # diff = t - loc (broadcast over K: 1x mode) in place;
                    # v/pw go to fresh tiles so DMA input tiles recycle fast
                    tb = t_all[:, sl].unsqueeze(2).broadcast_to([p, c, K])
                    nc.vector.tensor_sub(out=loc_t, in0=tb, in1=loc_t)
                    v_t = pv.tile([p, cmax, K], BF16, tag="v", name="vt")[:, :c, :]
                    nc.vector.tensor_mul(out=v_t, in0=loc_t, in1=rp_t)
                    pw_t = ppw.tile([p, cmax, K], BF16, tag="pw", name="pwt")[:, :c, :]
                    nc.vector.tensor_mul(out=pw_t, in0=rp_t, in1=w_t)   # rp*e^w
                    tree16(w_t, stash2[:, 0, sl])                       # den
                    binfo.append((sl, c, v_t, pw_t))
                return binfo

            def emit_B(binfo):
                # ---- phase B: tanh, (1-th^2), term, num tree ----
                for sl, c, v_t, pw_t in binfo:
                    act(out=v_t, in_=v_t, func=AF.Tanh, scale=0.5)      # th
                for sl, c, v_t, pw_t in binfo:
                    c1 = pc1.tile([p, cmax, K], BF16, tag="c1", name="c1t")[:, :c, :]
                    if c == cmax and sq_left[0] > 0:
                        # ACT square (Square is in every table set)
                        sq_left[0] -= 1
                        act(out=c1, in_=v_t, func=AF.Square)            # th^2
                        nc.vector.tensor_scalar(
                            out=c1, in0=c1, scalar1=-1.0, scalar2=1.0,
                            op0=OP.mult, op1=OP.add,
                        )                                               # 1-th^2
                    else:
                        # neg-then-mul keeps 2x (same-operand th*th is 1x)
                        nc.vector.tensor_scalar(
                            out=c1, in0=v_t, scalar1=-1.0, scalar2=None,
                            op0=OP.mult,
                        )                                               # -th
                        nc.vector.tensor_mul(out=c1, in0=c1, in1=v_t)   # -th^2
                        nc.vector.tensor_scalar(
                            out=c1, in0=c1, scalar1=1.0, scalar2=1.0,
                            op0=OP.mult, op1=OP.add,
                        )                                               # 1-th^2
                    nc.vector.tensor_mul(out=c1, in0=c1, in1=pw_t)      # term
                    tree16(c1, stash2[:, 1, sl])                        # num

            # --- software pipeline: supers ahead of their chunks, A(h+1)
            #     before B(h) ---------------------------------------------
            super_of_chunk = []
            for ci in range(len(chunks)):
                cs = starts[ci]
                for si, so in enumerate(su_starts):
                    if so <= cs < so + supers[si]:
                        super_of_chunk.append(si)
                        break

            emit_super(0)
            pending = None
            emitted_supers = 1
            for ci, ch in enumerate(chunks):
                need = super_of_chunk[ci + 1] if ci + 1 < len(chunks) else -1
                if super_of_chunk[ci] >= emitted_supers:
                    emit_super(super_of_chunk[ci])
                    emitted_supers += 1
                elif need >= emitted_supers:
                    # emit the next super's Lns one chunk early so its table
                    # switch rides the A-phase boundary
                    emit_super(need)
                    emitted_supers += 1
                binfo = emit_A(ci, ch)
                if pending is not None:
                    emit_B(pending)
                pending = binfo
            emit_B(pending)

            # ---- final: per-row logs + per-partition accumulation ----
            act(out=stash2[:, 1, :], in_=stash2[:, 1, :], func=AF.Ln,
                scale=0.25, accum_out=out_sb[:, 0:1])
            act(out=stash2[:, 0, :], in_=stash2[:, 0, :], func=AF.Ln,
                accum_out=out_sb[:, 1:2])
            nc.gpsimd.dma_start(out=out_d.ap(), in_=out_sb)

            # Pin ACT execution order (same engine -> scheduler-only edges)
            # so table-set switches stay at super/phase granularity.
            for prev, nxt in zip(acts, acts[1:]):
                add_dep_helper(nxt.ins, prev.ins, False, "act-table-order")

    nc.compile()
    return nc


def _combine(outs, n_rows):
    total = 0.0
    for o in outs:
        total += float(o[:, 0].sum(dtype=np.float64))
        total -= float(o[:, 1].sum(dtype=np.float64))
    return np.float32(total / n_rows)


def make_in_maps(weight, loc, scale, targets):
    w = np.ascontiguousarray(weight.reshape(N, K), dtype=np.float32)
    l = np.ascontiguousarray(loc.reshape(N, K), dtype=np.float32)
    s = np.ascontiguousarray(scale.reshape(N, K), dtype=np.float32)
    t = np.ascontiguousarray(targets.reshape(N), dtype=np.float32)
    in_maps = []
    for ci in range(NCORES):
        rs = slice(ci * NLOC, (ci + 1) * NLOC)
        in_maps.append({
            "w": np.ascontiguousarray(w[rs]),
            "loc": np.ascontiguousarray(l[rs]),
            "scale": np.ascontiguousarray(s[rs]),
            "t": np.ascontiguousarray(t[rs]),
        })
    return in_maps


def run(in_maps, **kwargs):
    nc = build_kernel()
    return run_bass_kernel_spmd(nc, in_maps, core_ids=list(range(NCORES)), **kwargs)


def kernel(weight, loc, scale, targets):
    in_maps = make_in_maps(weight, loc, scale, targets)
    last = None
    for _ in range(3):  # rare transient NRT device errors: retry
        try:
            res = run(in_maps)
            return _combine([r["out"] for r in res.results], N)
        except Exception as e:  # noqa: BLE001
            last = e
    raise last


if __name__ == "__main__":
    nc = build_kernel()
    print("kernel built OK")
